# revision 43
# baseline (speedup 1.0000x reference)
"""Trainium2 Bass kernel for nn_AttentionHead_46660524703758.

Reference computation (per batch b of 8):
    keys   = x @ kw            [2048, 64]
    values = x @ vw            [2048, 64]
    scores = keys @ values.T / 8          (masked: keep col >= row)
    out    = softmax(scores, -1) @ values

Sharding: data-parallel over the batch dim, one batch per NeuronCore (8 cores).

Per-core dataflow (all matmuls fp16 in / fp32 PSUM accumulate), pipelined in
four segment groups of 4 n-tiles each:
    x tiles --SWDGE cast DMA--> fp16
    x.T blocks via REGULAR matmul against identity (pipelines back-to-back on
        the PE and keeps the HAM clock-gate warm, unlike transpose-mode)
    xT seg --matmul [kw|vw] stationary--> [K^T; V^T] stacked
    V^T seg --SBUF DMA--> vT_lo (parts 0:64) --matmul transpose--> V nat (+ones)
    S^T[v,k] = V^T.T K^T  (upper blocks only) --ACT exp(s/8 - 6)--> P^T fp16
    [O^T; colsum] += [V|1].T stream P^T   (PSUM accumulate over v-chunks)
    O^T --PE transpose--> O natural; divide by colsum row; DMA out.

The exp bias of -6 keeps exp() inside fp16 range (softmax-invariant).
"""
import sys

if "/opt/trn_rl_repo" not in sys.path:
    sys.path.insert(0, "/opt/trn_rl_repo")

import numpy as np

import concourse.bacc as bacc
import concourse.mybir as mybir
from concourse import tile
from concourse.bass_utils import run_bass_kernel_spmd
from concourse.masks import make_identity

B, N, H, E = 8, 2048, 1024, 64
P = 128
NT = N // P   # 16 n-tiles
HC = H // P   # 8 h-chunks
DT = mybir.dt.float16
F32 = mybir.dt.float32
EXP_BIAS = -6.0
EXP_SCALE = 0.125  # 1/sqrt(E)
N_WARM = 24

_cached_nc = None


def build_nc():
    nc = bacc.Bacc("TRN2", target_bir_lowering=False, debug=False, num_devices=8)
    x = nc.dram_tensor("x", [N, H], F32, kind="ExternalInput")
    kwt = nc.dram_tensor("kw", [H, E], F32, kind="ExternalInput")
    vwt = nc.dram_tensor("vw", [H, E], F32, kind="ExternalInput")
    out = nc.dram_tensor("out", [N, E], F32, kind="ExternalOutput")

    Exp = mybir.ActivationFunctionType.Exp
    Copy = mybir.ActivationFunctionType.Copy

    with tile.TileContext(nc) as tc:
        with (
            tc.tile_pool(name="const", bufs=1) as const,
            tc.tile_pool(name="big", bufs=1) as big,
            tc.tile_pool(name="xin", bufs=16) as xin,
            tc.tile_pool(name="opool", bufs=2) as opool,
            tc.tile_pool(name="ps_st", bufs=4, space="PSUM") as ps_st,
            tc.tile_pool(name="ps_av", bufs=2, space="PSUM") as ps_av,
            tc.tile_pool(name="ps_tr", bufs=2, space="PSUM") as ps_tr,
        ):
            # --- x input DMAs (SWDGE cast fp32->fp16), all issued up front ---
            xts = []
            for t in range(NT):
                xt = xin.tile([P, H], DT, tag="xt", name=f"xt{t}")
                nc.gpsimd.dma_start(xt[:], x.ap()[t * P : (t + 1) * P, :])
                xts.append(xt)

            # --- constants, issued first so gpsimd/sync start instantly ---
            ident_h = const.tile([P, P], DT)
            make_identity(nc, ident_h[:])
            warm_src = const.tile([P, 512], DT)
            nc.gpsimd.memset(warm_src[:], 0.001)
            ident_f = const.tile([P, P], F32)
            make_identity(nc, ident_f[:])
            bias_t = const.tile([P, 1], F32)
            nc.gpsimd.memset(bias_t[:], EXP_BIAS)

            kw_sb = const.tile([P, HC * E], F32)
            vw_sb = const.tile([P, HC * E], F32)
            nc.sync.dma_start(
                kw_sb[:].rearrange("p (c e) -> p c e", c=HC),
                kwt.ap().rearrange("(c p) e -> p c e", p=P),
            )
            nc.sync.dma_start(
                vw_sb[:].rearrange("p (c e) -> p c e", c=HC),
                vwt.ap().rearrange("(c p) e -> p c e", p=P),
            )
            kvw = const.tile([P, HC, P], DT)
            nc.vector.tensor_copy(
                kvw[:, :, 0:E], kw_sb[:].rearrange("p (c e) -> p c e", c=HC)
            )
            nc.vector.tensor_copy(
                kvw[:, :, E:P], vw_sb[:].rearrange("p (c e) -> p c e", c=HC)
            )

            # --- PE warm-up: matmul bursts to trip/hold the HAM clock-gate ---
            warm_ctr = [0]

            def warm_burst(n):
                for _ in range(n):
                    wp = ps_tr.tile(
                        [P, 512], F32, tag="tr", name=f"warm{warm_ctr[0]}"
                    )
                    warm_ctr[0] += 1
                    nc.tensor.matmul(
                        wp[:], ident_h[:], warm_src[:], start=True, stop=True
                    )

            warm_burst(N_WARM)

            # --- main tensors ---
            xT = big.tile([P, HC * N], DT)  # [h_in_chunk, c*2048 + n]
            xT3 = xT[:].rearrange("p (c n) -> p c n", c=HC)
            kvT = big.tile([P, N], DT)      # rows 0:64 K^T, 64:128 V^T
            vT_lo = big.tile([64, N], DT)
            kT_hi = big.tile([P, N], DT)    # rows 64:128 hold a K^T copy
            vnat = big.tile([P, NT * (E + 1)], DT)
            vnat3 = vnat[:].rearrange("p (j c) -> p j c", c=E + 1)
            nc.gpsimd.memset(vnat3[:, :, E : E + 1], 1.0)
            pT = big.tile([P, NT * N], DT)  # [v_in_tile, i*2048 + k]
            av0 = ps_av.tile([P, 512], F32, tag="av", name="av0")

            # --- pipelined segment groups: 4 n-tiles -> proj seg -> S^T/AV ---
            for s in range(4):
                for t in range(4 * s, 4 * s + 4):
                    # x.T blocks via regular matmul (4 chunks per PSUM slot)
                    for half in range(2):
                        trp = ps_tr.tile(
                            [P, 512], F32, tag="tr", name=f"trp{t}_{half}"
                        )
                        for cc in range(4):
                            c = half * 4 + cc
                            nc.tensor.matmul(
                                trp[:, cc * P : (cc + 1) * P],
                                xts[t][:, c * P : (c + 1) * P],
                                ident_h[:],
                                start=True,
                                stop=True,
                            )
                        dst = xT3[:, half * 4 : half * 4 + 4, t * P : (t + 1) * P]
                        src = trp[:].rearrange("p (c n) -> p c n", c=4)
                        if s == 0:
                            nc.scalar.activation(dst, src, Copy)
                        else:
                            nc.vector.tensor_copy(dst, src)

                # projection segment: kvT cols [512s, 512s+512)
                ps = ps_st.tile([P, 512], F32, tag="st", name=f"proj{s}")
                for c in range(HC):
                    nc.tensor.matmul(
                        ps[:],
                        kvw[:, c, :],
                        xT3[:, c, s * 512 : (s + 1) * 512],
                        start=(c == 0),
                        stop=(c == HC - 1),
                    )
                nc.vector.tensor_copy(kvT[:, s * 512 : (s + 1) * 512], ps[:])
                nc.sync.dma_start(
                    vT_lo[:, s * 512 : (s + 1) * 512],
                    kvT[64:128, s * 512 : (s + 1) * 512],
                )
                nc.sync.dma_start(
                    kT_hi[64:128, s * 512 : (s + 1) * 512],
                    kvT[0:64, s * 512 : (s + 1) * 512],
                )
                vtr = ps_tr.tile([P, 4 * E], F32, tag="tr", name=f"vtr{s}")
                for jj in range(4):
                    j = 4 * s + jj
                    nc.tensor.matmul(
                        vtr[:, jj * E : (jj + 1) * E],
                        vT_lo[:, j * P : (j + 1) * P],
                        ident_h[0:64, 0:64],
                        start=True,
                        stop=True,
                    )
                nc.vector.tensor_copy(
                    vnat3[:, 4 * s : 4 * s + 4, 0:E],
                    vtr[:].rearrange("p (j e) -> p j e", j=4),
                )

                # S^T -> exp -> mask, two v-tiles packed per array pass
                # (row groups 0-1 for tile a at parts 0:64, 2-3 for tile b).
                for a in (4 * s, 4 * s + 2):
                    b = a + 1
                    kend_b = (b + 1) * P
                    for sc in range((kend_b + 511) // 512):
                        ks = sc * 512
                        nn_a = min(512, max(0, (a + 1) * P - ks))
                        nn_b = min(512, kend_b - ks)
                        if nn_a > 0:
                            sta = ps_st.tile(
                                [P, 512], F32, tag="st", name=f"st{a}_{sc}"
                            )
                            nc.tensor.matmul(
                                sta[:, 0:nn_a],
                                vT_lo[:, a * P : (a + 1) * P],
                                kvT[0:64, ks : ks + nn_a],
                                start=True,
                                stop=True,
                            )
                        stb = ps_st.tile([P, 512], F32, tag="st", name=f"st{b}_{sc}")
                        nc.tensor.matmul(
                            stb[:, 0:nn_b],
                            kvT[64:128, b * P : (b + 1) * P],
                            kT_hi[64:128, ks : ks + nn_b],
                            start=True,
                            stop=True,
                        )
                        if nn_a > 0:
                            nc.scalar.activation(
                                pT[:, a * N + ks : a * N + ks + nn_a],
                                sta[:, 0:nn_a],
                                Exp,
                                bias=bias_t[:],
                                scale=EXP_SCALE,
                            )
                        nc.scalar.activation(
                            pT[:, b * N + ks : b * N + ks + nn_b],
                            stb[:, 0:nn_b],
                            Exp,
                            bias=bias_t[:],
                            scale=EXP_SCALE,
                        )
                    for i in (a, b):
                        dg = i * N + i * P
                        nc.gpsimd.affine_select(
                            out=pT[:, dg : dg + P],
                            in_=pT[:, dg : dg + P],
                            compare_op=mybir.AluOpType.is_ge,
                            fill=0.0,
                            base=0,
                            pattern=[[-1, P]],
                            channel_multiplier=1,
                        )
                        nn0 = min(512, (i + 1) * P)
                        nc.tensor.matmul(
                            av0[0 : E + 1, 0:nn0],
                            vnat3[:, i, :],
                            pT[:, i * N : i * N + nn0],
                            start=(i == 0),
                            stop=(i == NT - 1),
                        )


            # --- AV phase: dense k-segment-major [O^T; colsum] accumulation ---
            out_sb = big.tile([P, NT * E], F32)
            for s in range(4):
                if s == 0:
                    av = av0
                else:
                    av = ps_av.tile([P, 512], F32, tag="av", name=f"avp{s}")
                    for i in range(4 * s, NT):
                        nn = min(512, (i + 1) * P - s * 512)
                        nc.tensor.matmul(
                            av[0 : E + 1, 0:nn],
                            vnat3[:, i, :],
                            pT[:, i * N + s * 512 : i * N + s * 512 + nn],
                            start=(i == s * 4),
                            stop=(i == NT - 1),
                        )
                oT = opool.tile([E + 1, 512], F32, tag="oT", name=f"oT{s}")
                nc.vector.tensor_copy(oT[:], av[0 : E + 1, :])
                for kt in range(4):
                    tr = ps_tr.tile([P, E + 1], F32, tag="tr", name=f"otr{s}_{kt}")
                    nc.tensor.transpose(
                        tr[:], oT[:, kt * P : (kt + 1) * P], ident_f[0 : E + 1, 0 : E + 1]
                    )
                    kti = s * 4 + kt
                    rec = opool.tile([P, 1], F32, tag="rec", name=f"rec{kti}")
                    nc.vector.reciprocal(rec[:], tr[:, E : E + 1])
                    nc.vector.tensor_scalar_mul(
                        out_sb[:, kti * E : (kti + 1) * E], tr[:, 0:E], rec[:]
                    )
                # store this k-segment as soon as its epilogue is done
                nc.sync.dma_start(
                    out.ap().rearrange("(t p) e -> p t e", p=P)[:, 4 * s : 4 * s + 4, :],
                    out_sb[:].rearrange("p (t e) -> p t e", t=NT)[
                        :, 4 * s : 4 * s + 4, :
                    ],
                )

    nc.finalize()
    return nc


def _get_nc():
    global _cached_nc
    if _cached_nc is None:
        _cached_nc = build_nc()
    return _cached_nc


def kernel(input, k, q, v, **extra_bass_kwargs):
    """Full-input entry point: shards batch across 8 cores, gathers output."""
    del q  # reference computes queries but never uses them
    input = np.ascontiguousarray(np.asarray(input, dtype=np.float32))
    k = np.ascontiguousarray(np.asarray(k, dtype=np.float32))
    v = np.ascontiguousarray(np.asarray(v, dtype=np.float32))
    nc = _get_nc()
    in_maps = [{"x": input[b], "kw": k, "vw": v} for b in range(B)]
    res = run_bass_kernel_spmd(
        nc, in_maps, core_ids=list(range(B)), **extra_bass_kwargs
    )
    out = np.stack([r["out"] for r in res.results]).astype(np.float32)
    if extra_bass_kwargs:
        kernel.last_results = res
    return out


# revision 44
# speedup vs baseline: 1.1472x; 1.1472x over previous
"""Trainium2 Bass kernel for nn_AttentionHead_46660524703758.

Reference computation (per batch b of 8):
    keys   = x @ kw            [2048, 64]
    values = x @ vw            [2048, 64]
    scores = keys @ values.T / 8          (masked: keep col >= row)
    out    = softmax(scores, -1) @ values

Sharding: data-parallel over the batch dim, one batch per NeuronCore (8 cores).

Per-core dataflow (all matmuls fp16 in / fp32 PSUM accumulate), pipelined in
four segment groups of 4 n-tiles each:
    x tiles --SWDGE cast DMA--> fp16
    x.T blocks via REGULAR matmul against identity (pipelines back-to-back on
        the PE and keeps the HAM clock-gate warm, unlike transpose-mode)
    xT seg --matmul [kw|vw] stationary--> [K^T; V^T] stacked
    V^T seg --SBUF DMA--> vT_lo (parts 0:64) --matmul transpose--> V nat (+ones)
    S^T[v,k] = V^T.T K^T  (upper blocks only) --ACT exp(s/8 - 6)--> P^T fp16
    [O^T; colsum] += [V|1].T stream P^T   (PSUM accumulate over v-chunks)
    O^T --PE transpose--> O natural; divide by colsum row; DMA out.

The exp bias of -6 keeps exp() inside fp16 range (softmax-invariant).
"""
import sys

if "/opt/trn_rl_repo" not in sys.path:
    sys.path.insert(0, "/opt/trn_rl_repo")

import numpy as np

import concourse.bacc as bacc
import concourse.mybir as mybir
from concourse import tile
from concourse.bass_utils import run_bass_kernel_spmd
from concourse.masks import make_identity

B, N, H, E = 8, 2048, 1024, 64
P = 128
NT = N // P   # 16 n-tiles
HC = H // P   # 8 h-chunks
DT = mybir.dt.float16
F32 = mybir.dt.float32
EXP_BIAS = -6.0
EXP_SCALE = 0.125  # 1/sqrt(E)
N_WARM = 24

_cached_nc = None


def build_nc():
    nc = bacc.Bacc("TRN2", target_bir_lowering=False, debug=False, num_devices=8)
    x = nc.dram_tensor("x", [N, H], F32, kind="ExternalInput")
    kwt = nc.dram_tensor("kw", [H, E], F32, kind="ExternalInput")
    vwt = nc.dram_tensor("vw", [H, E], F32, kind="ExternalInput")
    out = nc.dram_tensor("out", [N, E], F32, kind="ExternalOutput")

    Exp = mybir.ActivationFunctionType.Exp
    Copy = mybir.ActivationFunctionType.Copy

    with tile.TileContext(nc) as tc:
        with (
            tc.tile_pool(name="const", bufs=1) as const,
            tc.tile_pool(name="big", bufs=1) as big,
            tc.tile_pool(name="xin", bufs=16) as xin,
            tc.tile_pool(name="opool", bufs=2) as opool,
            tc.tile_pool(name="ps_st", bufs=4, space="PSUM") as ps_st,
            tc.tile_pool(name="ps_av", bufs=2, space="PSUM") as ps_av,
            tc.tile_pool(name="ps_tr", bufs=2, space="PSUM") as ps_tr,
        ):
            # --- constants, issued first so gpsimd/sync start instantly ---
            ident_h = const.tile([P, P], DT)
            make_identity(nc, ident_h[:])
            warm_src = const.tile([P, 512], DT)
            nc.gpsimd.memset(warm_src[:], 0.001)
            ident_f = const.tile([P, P], F32)
            make_identity(nc, ident_f[:])
            bias_t = const.tile([P, 1], F32)
            nc.gpsimd.memset(bias_t[:], EXP_BIAS)

            kw_sb = const.tile([P, HC * E], F32)
            vw_sb = const.tile([P, HC * E], F32)
            nc.sync.dma_start(
                kw_sb[:].rearrange("p (c e) -> p c e", c=HC),
                kwt.ap().rearrange("(c p) e -> p c e", p=P),
            )
            nc.sync.dma_start(
                vw_sb[:].rearrange("p (c e) -> p c e", c=HC),
                vwt.ap().rearrange("(c p) e -> p c e", p=P),
            )
            kvw = const.tile([P, HC, P], DT)
            nc.vector.tensor_copy(
                kvw[:, :, 0:E], kw_sb[:].rearrange("p (c e) -> p c e", c=HC)
            )
            nc.vector.tensor_copy(
                kvw[:, :, E:P], vw_sb[:].rearrange("p (c e) -> p c e", c=HC)
            )

            # --- x input DMAs (SWDGE cast fp32->fp16), all issued up front ---
            xts = []
            for t in range(NT):
                xt = xin.tile([P, H], DT, tag="xt", name=f"xt{t}")
                nc.gpsimd.dma_start(xt[:], x.ap()[t * P : (t + 1) * P, :])
                xts.append(xt)

            # --- PE warm-up: matmul bursts to trip/hold the HAM clock-gate ---
            warm_ctr = [0]

            def warm_burst(n):
                for _ in range(n):
                    wp = ps_tr.tile(
                        [P, 512], F32, tag="tr", name=f"warm{warm_ctr[0]}"
                    )
                    warm_ctr[0] += 1
                    nc.tensor.matmul(
                        wp[:], ident_h[:], warm_src[:], start=True, stop=True
                    )

            warm_burst(N_WARM)

            # --- main tensors ---
            xT = big.tile([P, HC * N], DT)  # [h_in_chunk, c*2048 + n]
            xT3 = xT[:].rearrange("p (c n) -> p c n", c=HC)
            kvT = big.tile([P, N], DT)      # rows 0:64 K^T, 64:128 V^T
            vT_lo = big.tile([64, N], DT)
            kT_hi = big.tile([P, N], DT)    # rows 64:128 hold a K^T copy
            vnat = big.tile([P, NT * (E + 1)], DT)
            vnat3 = vnat[:].rearrange("p (j c) -> p j c", c=E + 1)
            nc.gpsimd.memset(vnat3[:, :, E : E + 1], 1.0)
            pT = big.tile([P, NT * N], DT)  # [v_in_tile, i*2048 + k]
            av0 = ps_av.tile([P, 512], F32, tag="av", name="av0")

            # --- pipelined segment groups: 4 n-tiles -> proj seg -> S^T/AV ---
            for s in range(4):
                for t in range(4 * s, 4 * s + 4):
                    # x.T blocks via regular matmul (4 chunks per PSUM slot)
                    for half in range(2):
                        trp = ps_tr.tile(
                            [P, 512], F32, tag="tr", name=f"trp{t}_{half}"
                        )
                        for cc in range(4):
                            c = half * 4 + cc
                            nc.tensor.matmul(
                                trp[:, cc * P : (cc + 1) * P],
                                xts[t][:, c * P : (c + 1) * P],
                                ident_h[:],
                                start=True,
                                stop=True,
                            )
                        dst = xT3[:, half * 4 : half * 4 + 4, t * P : (t + 1) * P]
                        src = trp[:].rearrange("p (c n) -> p c n", c=4)
                        if s == 0:
                            nc.scalar.activation(dst, src, Copy)
                        else:
                            nc.vector.tensor_copy(dst, src)

                # projection segment: kvT cols [512s, 512s+512)
                ps = ps_st.tile([P, 512], F32, tag="st", name=f"proj{s}")
                for c in range(HC):
                    nc.tensor.matmul(
                        ps[:],
                        kvw[:, c, :],
                        xT3[:, c, s * 512 : (s + 1) * 512],
                        start=(c == 0),
                        stop=(c == HC - 1),
                    )
                nc.vector.tensor_copy(kvT[:, s * 512 : (s + 1) * 512], ps[:])
                nc.sync.dma_start(
                    vT_lo[:, s * 512 : (s + 1) * 512],
                    kvT[64:128, s * 512 : (s + 1) * 512],
                )
                nc.sync.dma_start(
                    kT_hi[64:128, s * 512 : (s + 1) * 512],
                    kvT[0:64, s * 512 : (s + 1) * 512],
                )
                vtr = ps_tr.tile([P, 4 * E], F32, tag="tr", name=f"vtr{s}")
                for jj in range(4):
                    j = 4 * s + jj
                    nc.tensor.matmul(
                        vtr[:, jj * E : (jj + 1) * E],
                        vT_lo[:, j * P : (j + 1) * P],
                        ident_h[0:64, 0:64],
                        start=True,
                        stop=True,
                    )
                nc.vector.tensor_copy(
                    vnat3[:, 4 * s : 4 * s + 4, 0:E],
                    vtr[:].rearrange("p (j e) -> p j e", j=4),
                )

                # S^T -> exp -> mask, two v-tiles packed per array pass
                # (row groups 0-1 for tile a at parts 0:64, 2-3 for tile b).
                for a in (4 * s, 4 * s + 2):
                    b = a + 1
                    kend_b = (b + 1) * P
                    for sc in range((kend_b + 511) // 512):
                        ks = sc * 512
                        nn_a = min(512, max(0, (a + 1) * P - ks))
                        nn_b = min(512, kend_b - ks)
                        if nn_a > 0:
                            sta = ps_st.tile(
                                [P, 512], F32, tag="st", name=f"st{a}_{sc}"
                            )
                            nc.tensor.matmul(
                                sta[:, 0:nn_a],
                                vT_lo[:, a * P : (a + 1) * P],
                                kvT[0:64, ks : ks + nn_a],
                                start=True,
                                stop=True,
                            )
                        stb = ps_st.tile([P, 512], F32, tag="st", name=f"st{b}_{sc}")
                        nc.tensor.matmul(
                            stb[:, 0:nn_b],
                            kvT[64:128, b * P : (b + 1) * P],
                            kT_hi[64:128, ks : ks + nn_b],
                            start=True,
                            stop=True,
                        )
                        if nn_a > 0:
                            nc.scalar.activation(
                                pT[:, a * N + ks : a * N + ks + nn_a],
                                sta[:, 0:nn_a],
                                Exp,
                                bias=bias_t[:],
                                scale=EXP_SCALE,
                            )
                        nc.scalar.activation(
                            pT[:, b * N + ks : b * N + ks + nn_b],
                            stb[:, 0:nn_b],
                            Exp,
                            bias=bias_t[:],
                            scale=EXP_SCALE,
                        )
                    for i in (a, b):
                        dg = i * N + i * P
                        nc.gpsimd.affine_select(
                            out=pT[:, dg : dg + P],
                            in_=pT[:, dg : dg + P],
                            compare_op=mybir.AluOpType.is_ge,
                            fill=0.0,
                            base=0,
                            pattern=[[-1, P]],
                            channel_multiplier=1,
                        )
                        nn0 = min(512, (i + 1) * P)
                        nc.tensor.matmul(
                            av0[0 : E + 1, 0:nn0],
                            vnat3[:, i, :],
                            pT[:, i * N : i * N + nn0],
                            start=(i == 0),
                            stop=(i == NT - 1),
                        )


            # --- AV phase: dense k-segment-major [O^T; colsum] accumulation ---
            out_sb = big.tile([P, NT * E], F32)
            for s in range(4):
                if s == 0:
                    av = av0
                else:
                    av = ps_av.tile([P, 512], F32, tag="av", name=f"avp{s}")
                    for i in range(4 * s, NT):
                        nn = min(512, (i + 1) * P - s * 512)
                        nc.tensor.matmul(
                            av[0 : E + 1, 0:nn],
                            vnat3[:, i, :],
                            pT[:, i * N + s * 512 : i * N + s * 512 + nn],
                            start=(i == s * 4),
                            stop=(i == NT - 1),
                        )
                oT = opool.tile([E + 1, 512], F32, tag="oT", name=f"oT{s}")
                nc.vector.tensor_copy(oT[:], av[0 : E + 1, :])
                for kt in range(4):
                    tr = ps_tr.tile([P, E + 1], F32, tag="tr", name=f"otr{s}_{kt}")
                    nc.tensor.transpose(
                        tr[:], oT[:, kt * P : (kt + 1) * P], ident_f[0 : E + 1, 0 : E + 1]
                    )
                    kti = s * 4 + kt
                    rec = opool.tile([P, 1], F32, tag="rec", name=f"rec{kti}")
                    nc.vector.reciprocal(rec[:], tr[:, E : E + 1])
                    nc.vector.tensor_scalar_mul(
                        out_sb[:, kti * E : (kti + 1) * E], tr[:, 0:E], rec[:]
                    )
                # store this k-segment as soon as its epilogue is done
                nc.sync.dma_start(
                    out.ap().rearrange("(t p) e -> p t e", p=P)[:, 4 * s : 4 * s + 4, :],
                    out_sb[:].rearrange("p (t e) -> p t e", t=NT)[
                        :, 4 * s : 4 * s + 4, :
                    ],
                )

    nc.finalize()
    return nc


def _get_nc():
    global _cached_nc
    if _cached_nc is None:
        _cached_nc = build_nc()
    return _cached_nc


def kernel(input, k, q, v, **extra_bass_kwargs):
    """Full-input entry point: shards batch across 8 cores, gathers output."""
    del q  # reference computes queries but never uses them
    input = np.ascontiguousarray(np.asarray(input, dtype=np.float32))
    k = np.ascontiguousarray(np.asarray(k, dtype=np.float32))
    v = np.ascontiguousarray(np.asarray(v, dtype=np.float32))
    nc = _get_nc()
    in_maps = [{"x": input[b], "kw": k, "vw": v} for b in range(B)]
    res = run_bass_kernel_spmd(
        nc, in_maps, core_ids=list(range(B)), **extra_bass_kwargs
    )
    out = np.stack([r["out"] for r in res.results]).astype(np.float32)
    if extra_bass_kwargs:
        kernel.last_results = res
    return out


# revision 45
# speedup vs baseline: 1.2263x; 1.0689x over previous
"""Trainium2 Bass kernel for nn_AttentionHead_46660524703758.

Reference computation (per batch b of 8):
    keys   = x @ kw            [2048, 64]
    values = x @ vw            [2048, 64]
    scores = keys @ values.T / 8          (masked: keep col >= row)
    out    = softmax(scores, -1) @ values

Sharding: data-parallel over the batch dim, one batch per NeuronCore (8 cores).

Per-core dataflow (all matmuls fp16 in / fp32 PSUM accumulate), pipelined in
four segment groups of 4 n-tiles each:
    x tiles --SWDGE cast DMA--> fp16
    x.T blocks via REGULAR matmul against identity (pipelines back-to-back on
        the PE and keeps the HAM clock-gate warm, unlike transpose-mode)
    xT seg --matmul [kw|vw] stationary--> [K^T; V^T] stacked
    V^T seg --SBUF DMA--> vT_lo (parts 0:64) --matmul transpose--> V nat (+ones)
    S^T[v,k] = V^T.T K^T  (upper blocks only) --ACT exp(s/8 - 6)--> P^T fp16
    [O^T; colsum] += [V|1].T stream P^T   (PSUM accumulate over v-chunks)
    O^T --PE transpose--> O natural; divide by colsum row; DMA out.

The exp bias of -6 keeps exp() inside fp16 range (softmax-invariant).
"""
import sys

if "/opt/trn_rl_repo" not in sys.path:
    sys.path.insert(0, "/opt/trn_rl_repo")

import numpy as np

import concourse.bacc as bacc
import concourse.mybir as mybir
from concourse import tile
from concourse.bass_utils import run_bass_kernel_spmd
from concourse.masks import make_identity

B, N, H, E = 8, 2048, 1024, 64
P = 128
NT = N // P   # 16 n-tiles
HC = H // P   # 8 h-chunks
DT = mybir.dt.float16
F32 = mybir.dt.float32
EXP_BIAS = -6.0
EXP_SCALE = 0.125  # 1/sqrt(E)
N_WARM = 24

_cached_nc = None


def build_nc():
    nc = bacc.Bacc("TRN2", target_bir_lowering=False, debug=False, num_devices=8)
    x = nc.dram_tensor("x", [N, H], F32, kind="ExternalInput")
    kwt = nc.dram_tensor("kw", [H, E], F32, kind="ExternalInput")
    vwt = nc.dram_tensor("vw", [H, E], F32, kind="ExternalInput")
    out = nc.dram_tensor("out", [N, E], F32, kind="ExternalOutput")

    Exp = mybir.ActivationFunctionType.Exp
    Copy = mybir.ActivationFunctionType.Copy

    with tile.TileContext(nc) as tc:
        with (
            tc.tile_pool(name="const", bufs=1) as const,
            tc.tile_pool(name="big", bufs=1) as big,
            tc.tile_pool(name="xin", bufs=16) as xin,
            tc.tile_pool(name="opool", bufs=2) as opool,
            tc.tile_pool(name="ps_st", bufs=2, space="PSUM") as ps_st,
            tc.tile_pool(name="ps_av", bufs=2, space="PSUM") as ps_av,
            tc.tile_pool(name="ps_tr", bufs=2, space="PSUM") as ps_tr,
        ):
            # --- constants, issued first so gpsimd/sync start instantly ---
            ident_h = const.tile([P, P], DT)
            make_identity(nc, ident_h[:])
            warm_src = const.tile([P, 512], DT)
            nc.gpsimd.memset(warm_src[:], 0.001)
            ident_f = const.tile([P, P], F32)
            make_identity(nc, ident_f[:])
            bias_t = const.tile([P, 1], F32)
            nc.gpsimd.memset(bias_t[:], EXP_BIAS)

            kw_sb = const.tile([P, HC * E], F32)
            vw_sb = const.tile([P, HC * E], F32)
            nc.sync.dma_start(
                kw_sb[:].rearrange("p (c e) -> p c e", c=HC),
                kwt.ap().rearrange("(c p) e -> p c e", p=P),
            )
            nc.sync.dma_start(
                vw_sb[:].rearrange("p (c e) -> p c e", c=HC),
                vwt.ap().rearrange("(c p) e -> p c e", p=P),
            )
            kvw = const.tile([P, HC, P], DT)
            nc.vector.tensor_copy(
                kvw[:, :, 0:E], kw_sb[:].rearrange("p (c e) -> p c e", c=HC)
            )
            nc.vector.tensor_copy(
                kvw[:, :, E:P], vw_sb[:].rearrange("p (c e) -> p c e", c=HC)
            )

            # --- x input DMAs (SWDGE cast fp32->fp16), all issued up front ---
            xts = []
            for t in range(NT):
                xt = xin.tile([P, H], DT, tag="xt", name=f"xt{t}")
                nc.gpsimd.dma_start(xt[:], x.ap()[t * P : (t + 1) * P, :])
                xts.append(xt)

            # --- PE warm-up: matmul bursts to trip/hold the HAM clock-gate ---
            warm_ctr = [0]

            def warm_burst(n):
                for _ in range(n):
                    wp = ps_tr.tile(
                        [P, 512], F32, tag="tr", name=f"warm{warm_ctr[0]}"
                    )
                    warm_ctr[0] += 1
                    nc.tensor.matmul(
                        wp[:], ident_h[:], warm_src[:], start=True, stop=True
                    )

            warm_burst(N_WARM)

            # --- main tensors ---
            xT = big.tile([P, HC * N], DT)  # [h_in_chunk, c*2048 + n]
            xT3 = xT[:].rearrange("p (c n) -> p c n", c=HC)
            kvT = big.tile([P, N], DT)      # rows 0:64 K^T, 64:128 V^T
            vT_lo = big.tile([64, N], DT)
            kT_hi = big.tile([P, N], DT)    # rows 64:128 hold a K^T copy
            vnat = big.tile([P, NT * (E + 1)], DT)
            vnat3 = vnat[:].rearrange("p (j c) -> p j c", c=E + 1)
            nc.gpsimd.memset(vnat3[:, :, E : E + 1], 1.0)
            pT = big.tile([P, NT * N], DT)  # [v_in_tile, i*2048 + k]
            av0 = ps_av.tile([P, 512], F32, tag="av", name="av0")

            # --- pipelined segment groups: 4 n-tiles -> proj seg -> S^T/AV ---
            for s in range(4):
                for t in range(4 * s, 4 * s + 4):
                    # x.T blocks via regular matmul (4 chunks per PSUM slot)
                    for half in range(2):
                        trp = ps_tr.tile(
                            [P, 512], F32, tag="tr", name=f"trp{t}_{half}"
                        )
                        for cc in range(4):
                            c = half * 4 + cc
                            nc.tensor.matmul(
                                trp[:, cc * P : (cc + 1) * P],
                                xts[t][:, c * P : (c + 1) * P],
                                ident_h[:],
                                start=True,
                                stop=True,
                            )
                        dst = xT3[:, half * 4 : half * 4 + 4, t * P : (t + 1) * P]
                        src = trp[:].rearrange("p (c n) -> p c n", c=4)
                        if s == 0:
                            nc.scalar.activation(dst, src, Copy)
                        else:
                            nc.vector.tensor_copy(dst, src)

                # projection segment: kvT cols [512s, 512s+512)
                ps = ps_st.tile([P, 512], F32, tag="st", name=f"proj{s}")
                for c in range(HC):
                    nc.tensor.matmul(
                        ps[:],
                        kvw[:, c, :],
                        xT3[:, c, s * 512 : (s + 1) * 512],
                        start=(c == 0),
                        stop=(c == HC - 1),
                    )
                nc.vector.tensor_copy(kvT[:, s * 512 : (s + 1) * 512], ps[:])
                nc.sync.dma_start(
                    vT_lo[:, s * 512 : (s + 1) * 512],
                    kvT[64:128, s * 512 : (s + 1) * 512],
                )
                nc.sync.dma_start(
                    kT_hi[64:128, s * 512 : (s + 1) * 512],
                    kvT[0:64, s * 512 : (s + 1) * 512],
                )
                vtr = ps_tr.tile([P, 4 * E], F32, tag="tr", name=f"vtr{s}")
                for jj in range(4):
                    j = 4 * s + jj
                    nc.tensor.matmul(
                        vtr[:, jj * E : (jj + 1) * E],
                        vT_lo[:, j * P : (j + 1) * P],
                        ident_h[0:64, 0:64],
                        start=True,
                        stop=True,
                    )
                nc.vector.tensor_copy(
                    vnat3[:, 4 * s : 4 * s + 4, 0:E],
                    vtr[:].rearrange("p (j e) -> p j e", j=4),
                )

                # S^T -> exp -> mask, two v-tiles packed per array pass
                # (row groups 0-1 for tile a at parts 0:64, 2-3 for tile b).
                pT3 = pT[:].rearrange("p (i k) -> p i k", i=NT)
                for a in (4 * s, 4 * s + 2):
                    b = a + 1
                    kend_b = (b + 1) * P
                    for sc in range((kend_b + 511) // 512):
                        ks = sc * 512
                        nn_a = min(512, max(0, (a + 1) * P - ks))
                        nn_b = min(512, kend_b - ks)
                        st2 = ps_st.tile([P, 1024], F32, tag="st", name=f"st{a}_{sc}")
                        if nn_a > 0:
                            nc.tensor.matmul(
                                st2[:, 0:nn_a],
                                vT_lo[:, a * P : (a + 1) * P],
                                kvT[0:64, ks : ks + nn_a],
                                start=True,
                                stop=True,
                            )
                        nc.tensor.matmul(
                            st2[:, 512 : 512 + nn_b],
                            kvT[64:128, b * P : (b + 1) * P],
                            kT_hi[64:128, ks : ks + nn_b],
                            start=True,
                            stop=True,
                        )
                        if nn_a == 512 and nn_b == 512:
                            # one ACTIVATE exps both chunks (strided output AP)
                            nc.scalar.activation(
                                pT3[:, a : b + 1, ks : ks + 512],
                                st2[:].rearrange("p (two k) -> p two k", two=2),
                                Exp,
                                bias=bias_t[:],
                                scale=EXP_SCALE,
                            )
                        else:
                            if nn_a > 0:
                                nc.scalar.activation(
                                    pT[:, a * N + ks : a * N + ks + nn_a],
                                    st2[:, 0:nn_a],
                                    Exp,
                                    bias=bias_t[:],
                                    scale=EXP_SCALE,
                                )
                            nc.scalar.activation(
                                pT[:, b * N + ks : b * N + ks + nn_b],
                                st2[:, 512 : 512 + nn_b],
                                Exp,
                                bias=bias_t[:],
                                scale=EXP_SCALE,
                            )
                    for i in (a, b):
                        dg = i * N + i * P
                        nc.gpsimd.affine_select(
                            out=pT[:, dg : dg + P],
                            in_=pT[:, dg : dg + P],
                            compare_op=mybir.AluOpType.is_ge,
                            fill=0.0,
                            base=0,
                            pattern=[[-1, P]],
                            channel_multiplier=1,
                        )
                        nn0 = min(512, (i + 1) * P)
                        nc.tensor.matmul(
                            av0[0 : E + 1, 0:nn0],
                            vnat3[:, i, :],
                            pT[:, i * N : i * N + nn0],
                            start=(i == 0),
                            stop=(i == NT - 1),
                        )


            # --- AV phase: dense k-segment-major [O^T; colsum] accumulation ---
            out_sb = big.tile([P, NT * E], F32)
            for s in range(4):
                if s == 0:
                    av = av0
                else:
                    av = ps_av.tile([P, 512], F32, tag="av", name=f"avp{s}")
                    for i in range(4 * s, NT):
                        nn = min(512, (i + 1) * P - s * 512)
                        nc.tensor.matmul(
                            av[0 : E + 1, 0:nn],
                            vnat3[:, i, :],
                            pT[:, i * N + s * 512 : i * N + s * 512 + nn],
                            start=(i == s * 4),
                            stop=(i == NT - 1),
                        )
                oT = opool.tile([E + 1, 512], F32, tag="oT", name=f"oT{s}")
                nc.vector.tensor_copy(oT[:], av[0 : E + 1, :])
                for kt in range(4):
                    tr = ps_tr.tile([P, E + 1], F32, tag="tr", name=f"otr{s}_{kt}")
                    nc.tensor.transpose(
                        tr[:], oT[:, kt * P : (kt + 1) * P], ident_f[0 : E + 1, 0 : E + 1]
                    )
                    kti = s * 4 + kt
                    rec = opool.tile([P, 1], F32, tag="rec", name=f"rec{kti}")
                    nc.vector.reciprocal(rec[:], tr[:, E : E + 1])
                    nc.vector.tensor_scalar_mul(
                        out_sb[:, kti * E : (kti + 1) * E], tr[:, 0:E], rec[:]
                    )
                # store this k-segment as soon as its epilogue is done
                nc.sync.dma_start(
                    out.ap().rearrange("(t p) e -> p t e", p=P)[:, 4 * s : 4 * s + 4, :],
                    out_sb[:].rearrange("p (t e) -> p t e", t=NT)[
                        :, 4 * s : 4 * s + 4, :
                    ],
                )

    nc.finalize()
    return nc


def _get_nc():
    global _cached_nc
    if _cached_nc is None:
        _cached_nc = build_nc()
    return _cached_nc


def kernel(input, k, q, v, **extra_bass_kwargs):
    """Full-input entry point: shards batch across 8 cores, gathers output."""
    del q  # reference computes queries but never uses them
    input = np.ascontiguousarray(np.asarray(input, dtype=np.float32))
    k = np.ascontiguousarray(np.asarray(k, dtype=np.float32))
    v = np.ascontiguousarray(np.asarray(v, dtype=np.float32))
    nc = _get_nc()
    in_maps = [{"x": input[b], "kw": k, "vw": v} for b in range(B)]
    res = run_bass_kernel_spmd(
        nc, in_maps, core_ids=list(range(B)), **extra_bass_kwargs
    )
    out = np.stack([r["out"] for r in res.results]).astype(np.float32)
    if extra_bass_kwargs:
        kernel.last_results = res
    return out
